# revision 8
# baseline (speedup 1.0000x reference)
"""Trainium2 Bass kernel v2 for differentiable rotated-box IoU.

Full inputs: box1, box2 [4, 131072, 5] f32 (x, y, w, h, alpha).
Output: IoU [4, 131072] f32.

Green's-theorem / Liang-Barsky rotated-IoU, engineered for the TRN2
DVE perf-mode table:
  - bulk quad math in fp16 (DVE tensor_tensor 2x_1p mode, 2x faster)
  - reciprocals stay fp32 (HW seed trick requires it), output clamped to
    +-4096 while converting to fp16 (keeps every downstream fp16 value
    finite: |m| <= 5.5*4096, |w| <= 2*4096, sums < 2^15 -> no NaN)
  - pass-1 cross terms evaluated in box1's axis frame (cross products are
    rotation-invariant): cross_k = w1*h1/2 +- {w1*t'y, h1*t'x}, linear in
    per-pair scalars -- replaces per-edge explicit cross products
  - edge directions built at HALF magnitude; recip(d/2) = 2/d doubles all
    t-values uniformly, so interval clamps become (0, 2) and the final
    area scale becomes 0.25 -- the 0.5 factors fold into constants
  - work split across DVE / Pool(gpsimd) / Act(scalar) explicitly
  - two half-size chunks emitted stage-interleaved (engine queues are
    in-order; interleaving two chunks x two passes keeps every engine fed)

Sharding: data-parallel over 4*131072 = 524288 pairs, 65536 per core,
[128 partitions x 512 free] per core.
"""

import os
import sys

import numpy as np

if "/opt/trn_rl_repo" not in sys.path:
    sys.path.insert(0, "/opt/trn_rl_repo")

import concourse.bass as bass
import concourse.bacc as bacc
import concourse.mybir as mybir
from concourse.bass_utils import run_bass_kernel_spmd
from concourse.tile import TileContext

F32 = mybir.dt.float32
F16 = mybir.dt.float16
OP = mybir.AluOpType
AF = mybir.ActivationFunctionType

NCORES = 8
P = 128
S = 65536            # box pairs per core
FTOT = S // P        # 512
NCHUNK = int(os.environ.get("KNCHUNK", "2"))
CLAMP = 4096.0
PI = float(np.pi)

_CACHE = {}
LAST_RESULTS = None

# flexible op -> engine (V=DVE, G=Pool); override: KASSIGN="nl8:G,dt:V"
ASSIGN = {"uv8m": "V", "nl8": "V", "hi8": "V", "dt": "V",
          "crossq": "V", "red": "V", "cp": "V", "dqh": "G"}
for _kv in os.environ.get("KASSIGN", "").split(","):
    if ":" in _kv:
        _k, _v = _kv.split(":")
        ASSIGN[_k.strip()] = _v.strip()


def _build_program(nchunk=NCHUNK):
    F = FTOT // nchunk
    RW = F * 5

    nc = bacc.Bacc("TRN2", target_bir_lowering=False, debug=False,
                   num_devices=NCORES)

    # register the pi/2 constant used as activation bias for cos-via-sin
    _ct = nc.alloc_sbuf_tensor("const-f32-halfpi", [128, 1], F32)
    nc.gpsimd.memset(_ct.ap(), PI / 2)
    nc.const_aps.aps[(F32, PI / 2)] = _ct.ap()
    nc.all_engine_barrier()

    b1 = nc.dram_tensor("b1", [S, 5], F32, kind="ExternalInput")
    b2 = nc.dram_tensor("b2", [S, 5], F32, kind="ExternalInput")
    iou = nc.dram_tensor("iou", [S], F32, kind="ExternalOutput")

    b1v = b1.ap().flatten().rearrange("(p q) -> p q", p=P)
    b2v = b2.ap().flatten().rearrange("(p q) -> p q", p=P)
    iouv = iou.ap().rearrange("(p q) -> p q", p=P)

    bufs = nchunk
    repeat = int(os.environ.get("KREPEAT", "1"))
    with TileContext(nc) as tc:
        with tc.tile_pool(name="rawp", bufs=bufs + 1) as rawp, \
             tc.tile_pool(name="pool", bufs=bufs) as pool:
            def emit_all():
                per_chunk = [_chunk_stages(nc, rawp, pool, b1v, b2v, iouv,
                                           c, F, RW)
                             for c in range(nchunk)]
                nst = len(per_chunk[0])
                offset = max(1, nst // nchunk)
                for i in range(nst + offset * (nchunk - 1)):
                    for ci in range(nchunk):
                        j = i - offset * ci
                        if 0 <= j < nst:
                            per_chunk[ci][j]()
            if repeat > 1:
                unroll = int(os.environ.get("KUNROLL", "8"))
                while repeat % unroll:
                    unroll -= 1
                with tc.For_i(0, repeat // unroll, 1, staggered_reset=True):
                    for _ in range(unroll):
                        emit_all()
            else:
                emit_all()
    nc.compile()
    return nc


def _chunk_stages(nc, rawp, pool, b1v, b2v, iouv, c, F, RW):
    """Build the chunk's instruction stream as a list of stage closures."""
    V, G, A = nc.vector, nc.gpsimd, nc.scalar
    sfx = f"_{c}"

    def t16(name, tag=None, w=1):
        return pool.tile([P, w * F], F16, name=name + sfx, tag=(tag or name))

    def t32(name, tag=None, w=1):
        return pool.tile([P, w * F], F32, name=name + sfx, tag=(tag or name))

    def rep(t, width):
        # [t | t] broadcast of a [P, width] AP, viewed [P, 2, width]
        return bass.AP(t.tensor, t.offset, [t.ap[0], [0, 2], [1, width]])

    def q(t):
        return t.rearrange("p (r f) -> p r f", r=2)

    class NS:
        pass

    n = NS()          # chunk-level tensors
    F2, F3, F4 = 2 * F, 3 * F, 4 * F

    class Ctx:
        pass

    def mk(pref, sgn1, sgn2, want_cross):
        o = Ctx()
        o.pref, o.sgn1, o.sgn2 = pref, sgn1, sgn2
        o.want_cross = want_cross
        return o

    # pass 1: box1 edges vs box2 (sr signs +,-); pass 2: signs -,+
    p1 = mk("p1", +1.0, -1.0, True)
    p2 = mk("p2", -1.0, +1.0, False)
    passes = [p1, p2]

    # ---------------- chunk-level stages -------------------------------
    def s_dma():
        n.raw1 = rawp.tile([P, RW], F32, name="raw1" + sfx, tag="raw1")
        n.raw2 = rawp.tile([P, RW], F32, name="raw2" + sfx, tag="raw2")
        nc.sync.dma_start(n.raw1[:], b1v[:, c * RW:(c + 1) * RW])
        nc.sync.dma_start(n.raw2[:], b2v[:, c * RW:(c + 1) * RW])
        n.x1, n.y1, n.w1, n.h1, n.a1 = (n.raw1[:, i:RW:5] for i in range(5))
        n.x2, n.y2, n.w2, n.h2, n.a2 = (n.raw2[:, i:RW:5] for i in range(5))
        p1.wa, p1.ha = n.w1, n.h1
        p2.wa, p2.ha = n.w2, n.h2

    def s_trig():
        n.da = t32("da")
        G.tensor_sub(n.da, n.a1, n.a2)
        n.sr = t32("sr")
        A.activation(n.sr, n.da, AF.Sin)
        n.ada = t32("ada")
        A.activation(n.ada, n.da, AF.Abs)
        n.cr = t32("cr")
        A.activation(n.cr, n.ada, AF.Sin, bias=PI / 2, scale=-1.0)  # cos(da)
        n.s2 = t16("s2")
        A.activation(n.s2, n.a2, AF.Sin)
        n.c2 = t16("c2")
        A.activation(n.c2, n.a2, AF.Sin, bias=PI / 2, scale=-1.0)   # cos(a2)
        n.s1 = t16("s1")
        A.activation(n.s1, n.a1, AF.Sin)
        n.c1 = t16("c1")
        A.activation(n.c1, n.a1, AF.Sin, bias=PI / 2, scale=-1.0)   # cos(a1)
        # pre-scaled trig (Pool has no scalar ops; fold signs/halves here)
        n.ncrh = t32("ncrh")
        A.activation(n.ncrh, n.cr, AF.Copy, scale=-0.5)
        n.psrh = t32("psrh")
        A.activation(n.psrh, n.sr, AF.Copy, scale=0.5)
        n.nsrh = t32("nsrh")
        A.activation(n.nsrh, n.sr, AF.Copy, scale=-0.5)

    def s_diff():
        n.dxp = t32("dxp")
        G.tensor_sub(n.dxp, n.x1, n.x2)
        n.dyp = t32("dyp")
        G.tensor_sub(n.dyp, n.y1, n.y2)
        n.dx16 = t16("dx16")
        V.tensor_copy(n.dx16, n.dxp)
        n.dy16 = t16("dy16")
        V.tensor_copy(n.dy16, n.dyp)
        # t = R2^T (c1-c2): box1 center in box2 frame (fp16, DVE 2x)
        n.e1 = t16("e1")
        V.tensor_mul(n.e1, n.dx16, n.c2)
        n.e2 = t16("e2")
        V.tensor_mul(n.e2, n.dy16, n.s2)
        n.txty = t16("txty", w=2)
        n.tx = n.txty[:, :F]
        V.tensor_add(n.tx, n.e1, n.e2)
        n.e3 = t16("e3", "e1")
        V.tensor_mul(n.e3, n.dy16, n.c2)
        n.e4 = t16("e4", "e2")
        V.tensor_mul(n.e4, n.dx16, n.s2)
        n.ty = n.txty[:, F:]
        V.tensor_sub(n.ty, n.e3, n.e4)

    def s_t2():
        # t2 = R1^T (c2-c1): t2x = -(dx*c1 + dy*s1); t2y = dx*s1 - dy*c1
        n.g1 = t16("g1", "e1")
        V.tensor_mul(n.g1, n.dx16, n.c1)
        n.g2 = t16("g2", "e2")
        V.tensor_mul(n.g2, n.dy16, n.s1)
        n.ng = t16("ng")                 # ng = t'x = dx*c1 + dy*s1
        V.tensor_add(n.ng, n.g1, n.g2)
        n.t2xy = t16("t2xy", w=2)
        n.t2x = n.t2xy[:, :F]
        V.tensor_scalar(out=n.t2x, in0=n.ng, scalar1=-1.0, scalar2=None,
                        op0=OP.mult)
        n.g3 = t16("g3", "e1")
        V.tensor_mul(n.g3, n.dx16, n.s1)
        n.g4 = t16("g4", "e2")
        V.tensor_mul(n.g4, n.dy16, n.c1)
        n.t2y = n.t2xy[:, F:]
        V.tensor_sub(n.t2y, n.g3, n.g4)  # t2y = -t'y
        n.t2yn = t16("t2yn")
        V.tensor_sub(n.t2yn, n.g4, n.g3)  # t'y

    def s_areas():
        n.area1 = t32("area1")
        G.tensor_mul(n.area1, n.w1, n.h1)
        n.area2 = t32("area2")
        G.tensor_mul(n.area2, n.w2, n.h2)
        n.ssum = t32("ssum")
        G.tensor_add(n.ssum, n.area1, n.area2)
        n.K1 = t16("K1")
        A.activation(n.K1, n.area1, AF.Copy, scale=0.5)
        n.ha2 = t16("ha2")
        A.activation(n.ha2, n.area2, AF.Copy, scale=0.5)
        # clip-box half extents, packed [0.5*w | 0.5*h] fp16
        n.wb2p = t16("wb2p", w=2)
        A.activation(n.wb2p[:, :F], n.w2, AF.Copy, scale=0.5)
        A.activation(n.wb2p[:, F:], n.h2, AF.Copy, scale=0.5)
        n.wb1p = t16("wb1p", w=2)
        A.activation(n.wb1p[:, :F], n.w1, AF.Copy, scale=0.5)
        A.activation(n.wb1p[:, F:], n.h1, AF.Copy, scale=0.5)
        # pass-1 cross linear terms: uv = [w1*t'y | -h1*t'x] fp16
        n.uvh = t16("uvh", w=2)
        V.tensor_mul(n.uvh[:, :F], n.t2yn, n.wb1p[:, :F])   # 0.5*w1*t'y
        V.tensor_mul(n.uvh[:, F:], n.t2x, n.wb1p[:, F:])    # -0.5*h1*t'x
        n.uv = t16("uv", w=2)
        V.tensor_scalar(out=n.uv, in0=n.uvh, scalar1=2.0, scalar2=None,
                        op0=OP.mult)
        p1.txty = n.txty
        p2.txty = n.t2xy
        p1.wbp = n.wb2p
        p2.wbp = n.wb1p

    # ---------------- per-pass stages ----------------------------------
    def stage_dqh(o):
        o.dQh = t32(o.pref + "dQh", o.pref + "dQh", w=4)
        s1t = n.psrh if o.sgn1 > 0 else n.nsrh
        s2t_ = n.psrh if o.sgn2 > 0 else n.nsrh
        ed = G if ASSIGN.get("dqh", "V") == "G" else V
        ed.tensor_mul(o.dQh[:, :F], n.ncrh, o.wa)
        ed.tensor_mul(o.dQh[:, F:F2], s1t, o.ha)
        ed.tensor_mul(o.dQh[:, F2:F3], s2t_, o.wa)
        ed.tensor_mul(o.dQh[:, F3:], n.ncrh, o.ha)

    def stage_recip(o):
        o.rQf = t32(o.pref + "rQf", o.pref + "rQf", w=4)
        V.reciprocal_approx_fast(out=o.rQf, in_=o.dQh)

    def stage_pq(o):
        nm = o.pref
        # PQuv = [P0 | nP1 | Q0 | nQ1]; P0 = -(d0+d1), Q0 = -(d2+d3)
        # Pool has no scalar/negate ops: compute the sums, negate on DVE.
        o.PQuv = t16(nm + "PQuv", nm + "tA", w=4)
        o.PQn2 = t16(nm + "PQn2", nm + "PQn2", w=2)
        V.tensor_add(o.PQn2[:, :F], o.dQh[:, :F], o.dQh[:, F:F2])
        V.tensor_sub(o.PQuv[:, F:F2], o.dQh[:, :F], o.dQh[:, F:F2])
        V.tensor_add(o.PQn2[:, F:], o.dQh[:, F2:F3], o.dQh[:, F3:])
        V.tensor_sub(o.PQuv[:, F3:], o.dQh[:, F2:F3], o.dQh[:, F3:])
        slot02 = bass.AP(o.PQuv.tensor, o.PQuv.offset,
                         [o.PQuv.ap[0], [F2, 2], [1, F]])
        V.tensor_scalar(out=slot02, in0=q(o.PQn2), scalar1=-1.0,
                        scalar2=None, op0=OP.mult)

    def stage_clamp(o):
        o.rq = t16(o.pref + "rq", o.pref + "tB", w=4)
        V.tensor_scalar(out=o.rq, in0=o.rQf, scalar1=CLAMP, scalar2=-CLAMP,
                        op0=OP.min, op1=OP.max)

    def stage_ptq(o):
        o.ptQ = t16(o.pref + "ptQ", o.pref + "tC", w=4)
        wbrep = bass.AP(o.wbp.tensor, o.wbp.offset,
                        [o.wbp.ap[0], [F, 2], [0, 2], [1, F]])
        V.tensor_tensor(o.ptQ.rearrange("p (j r f) -> p j r f", j=2, r=2),
                        wbrep,
                        o.rq.rearrange("p (j r f) -> p j r f", j=2, r=2),
                        OP.mult)

    def stage_wq(o):
        o.wQ = t16(o.pref + "wQ", o.pref + "tC", w=4)
        A.activation(o.wQ, o.ptQ, AF.Abs)

    def stage_uvq(o):
        nm = o.pref
        # UV8 = [PQuv + (tx,tx,ty,ty) | PQuv - (tx,tx,ty,ty)]
        # blocks (2F each): [u01 | v01 | -u23 | -v23]
        o.UV8 = t16(nm + "UV8", nm + "o8A", w=8)
        tt = o.txty
        ttrep = bass.AP(tt.tensor, tt.offset,
                        [tt.ap[0], [F, 2], [0, 2], [1, F]])
        qv = "p (j r f) -> p j r f"
        V.tensor_tensor(o.UV8[:, :F4].rearrange(qv, j=2, r=2),
                        o.PQuv.rearrange(qv, j=2, r=2), ttrep, OP.add)
        e = G if ASSIGN["uv8m"] == "G" else V
        e.tensor_tensor(o.UV8[:, F4:].rearrange(qv, j=2, r=2),
                        o.PQuv.rearrange(qv, j=2, r=2), ttrep, OP.subtract)

    def stage_mxy(o):
        nm = o.pref
        # m8 = UV8 * [ru | rv | ru | rv]
        o.m8 = t16(nm + "m8", nm + "o8B", w=8)
        V.tensor_tensor(q(o.m8), q(o.UV8), rep(o.rq, F4), OP.mult)

    def stage_nlhi(o):
        nm = o.pref
        # nl8 = m8 + [wQ | wQ]; hi8 = [wQ | wQ] - m8
        o.nl8 = t16(nm + "nl8", nm + "o8A", w=8)
        e1_ = G if ASSIGN["nl8"] == "G" else V
        e1_.tensor_tensor(q(o.nl8), q(o.m8), rep(o.wQ, F4), OP.add)
        o.hi8 = t16(nm + "hi8", nm + "o8B", w=8)
        e2_ = G if ASSIGN["hi8"] == "G" else V
        e2_.tensor_tensor(q(o.hi8), rep(o.wQ, F4), q(o.m8), OP.subtract)

    def stage_z(o):
        nm = o.pref
        # m1 = min(nl_u, nl_v); m2 = min(hi_u, hi_v)  (V: Pool min illegal)
        def blocks(t8, first):
            off = 0 if first else F2
            return bass.AP(t8.tensor, t8.offset + off,
                           [t8.ap[0], [F4, 2], [1, F2]])
        o.m1 = t16(nm + "m1", nm + "tD", w=4)
        V.tensor_tensor(o.m1.rearrange("p (r f) -> p r f", r=2),
                        blocks(o.nl8, True), blocks(o.nl8, False), OP.min)
        o.m2 = t16(nm + "m2", nm + "tE", w=4)
        V.tensor_tensor(o.m2.rearrange("p (r f) -> p r f", r=2),
                        blocks(o.hi8, True), blocks(o.hi8, False), OP.min)
        # z1 = min(m1, 0); z2 = min(m2, 2)   [t doubled]
        o.z1 = t16(nm + "z1", nm + "tA", w=4)
        V.tensor_scalar(out=o.z1, in0=o.m1, scalar1=0.0, scalar2=None,
                        op0=OP.min)
        o.z2 = t16(nm + "z2", nm + "tB", w=4)
        V.tensor_scalar(out=o.z2, in0=o.m2, scalar1=2.0, scalar2=None,
                        op0=OP.min)

    def stage_dt(o):
        o.dt = t16(o.pref + "dt", o.pref + "tC", w=4)
        e = G if ASSIGN["dt"] == "G" else V
        e.tensor_add(o.dt, o.z1, o.z2)

    def stage_tail(o):
        nm = o.pref
        ec = G if ASSIGN["crossq"] == "G" else V
        er = G if ASSIGN["red"] == "G" else V
        rdt = t16(nm + "rdt", nm + "tC", w=4)
        V.tensor_scalar(out=rdt, in0=o.dt, scalar1=0.0, scalar2=None,
                        op0=OP.max)
        if o.want_cross:
            crossQ = t16(nm + "crossQ", nm + "tA", w=4)
            ec.tensor_tensor(q(crossQ[:, :F2]), q(n.uv), rep(n.K1, F), OP.add)
            ec.tensor_tensor(q(crossQ[:, F2:]), rep(n.K1, F), q(n.uv),
                             OP.subtract)
            cc = t16(nm + "cc", nm + "tB", w=4)
            V.tensor_mul(cc, rdt, crossQ)
            s = t16(nm + "s", nm + "s", w=2)
            er.tensor_add(s, cc[:, :F2], cc[:, F2:])
            o.out = t16(nm + "accp", nm + "accp")
            er.tensor_add(o.out, s[:, :F], s[:, F:])
        else:
            s2t = t16(nm + "s2t", nm + "s2t", w=2)
            er.tensor_add(s2t, rdt[:, :F2], rdt[:, F2:])
            o.out = t16(nm + "sdt", nm + "sdt")
            er.tensor_add(o.out, s2t[:, :F], s2t[:, F:])

    def s_combine():
        accp1, sdt = p1.out, p2.out
        cp2 = t16("cp2")
        V.tensor_mul(cp2, sdt, n.ha2)
        acc = t16("acc")
        V.tensor_add(acc, accp1, cp2)
        inter = t32("inter")
        A.activation(inter, acc, AF.Abs, scale=0.25)
        union = t32("union", "ng")
        G.tensor_sub(union, n.ssum, inter)
        runion = t32("runion", "da")
        V.reciprocal_approx_fast(out=runion, in_=union)
        iouT = t32("iouT", "area1")
        G.tensor_mul(iouT, inter, runion)
        outq = os.environ.get("KOUT", "sp")
        if c == 0 or outq == "sp":
            nc.sync.dma_start(iouv[:, c * F:(c + 1) * F], iouT)
        else:
            G.dma_start(iouv[:, c * F:(c + 1) * F], iouT)

    def both(fn):
        def go():
            for o in passes:
                fn(o)
        return go

    return [s_dma, s_trig,
            both(stage_dqh), both(stage_recip),
            s_diff, both(stage_pq), both(stage_clamp),
            s_t2, s_areas,
            both(stage_ptq), both(stage_wq),
            both(stage_uvq), both(stage_mxy), both(stage_nlhi),
            both(stage_z), both(stage_dt), both(stage_tail),
            s_combine]


def _get_program():
    key = ("prog", NCHUNK, os.environ.get("KREPEAT", "1"),
           os.environ.get("KUNROLL", "8"), os.environ.get("KOUT", "mix"),
           os.environ.get("KASSIGN", ""))
    if key not in _CACHE:
        _CACHE[key] = _build_program(NCHUNK)
    return _CACHE[key]


def kernel(box1, box2, trace=False):
    global LAST_RESULTS
    b1 = np.ascontiguousarray(np.asarray(box1, dtype=np.float32))
    b2 = np.ascontiguousarray(np.asarray(box2, dtype=np.float32))
    B, N, C = b1.shape
    T = B * N
    assert T == NCORES * S and C == 5, (b1.shape,)
    b1f = b1.reshape(T, 5)
    b2f = b2.reshape(T, 5)

    in_maps = [
        {"b1": b1f[i * S:(i + 1) * S], "b2": b2f[i * S:(i + 1) * S]}
        for i in range(NCORES)
    ]
    nc = _get_program()
    res = run_bass_kernel_spmd(nc, in_maps, list(range(NCORES)), trace=trace)
    LAST_RESULTS = res
    out = np.concatenate([res.results[i]["iou"] for i in range(NCORES)])
    return out.reshape(B, N)


# revision 10
# speedup vs baseline: 1.0971x; 1.0971x over previous
"""Trainium2 Bass kernel v2 for differentiable rotated-box IoU.

Full inputs: box1, box2 [4, 131072, 5] f32 (x, y, w, h, alpha).
Output: IoU [4, 131072] f32.

Green's-theorem / Liang-Barsky rotated-IoU, engineered for the TRN2
DVE perf-mode table:
  - bulk quad math in fp16 (DVE tensor_tensor 2x_1p mode, 2x faster)
  - reciprocals stay fp32 (HW seed trick requires it), output clamped to
    +-4096 while converting to fp16 (keeps every downstream fp16 value
    finite: |m| <= 5.5*4096, |w| <= 2*4096, sums < 2^15 -> no NaN)
  - pass-1 cross terms evaluated in box1's axis frame (cross products are
    rotation-invariant): cross_k = w1*h1/2 +- {w1*t'y, h1*t'x}, linear in
    per-pair scalars -- replaces per-edge explicit cross products
  - edge directions built at HALF magnitude; recip(d/2) = 2/d doubles all
    t-values uniformly, so interval clamps become (0, 2) and the final
    area scale becomes 0.25 -- the 0.5 factors fold into constants
  - work split across DVE / Pool(gpsimd) / Act(scalar) explicitly
  - two half-size chunks emitted stage-interleaved (engine queues are
    in-order; interleaving two chunks x two passes keeps every engine fed)

Sharding: data-parallel over 4*131072 = 524288 pairs, 65536 per core,
[128 partitions x 512 free] per core.
"""

import os
import sys

import numpy as np

if "/opt/trn_rl_repo" not in sys.path:
    sys.path.insert(0, "/opt/trn_rl_repo")

import concourse.bass as bass
import concourse.bacc as bacc
import concourse.mybir as mybir
from concourse.bass_utils import run_bass_kernel_spmd
from concourse.tile import TileContext

F32 = mybir.dt.float32
F16 = mybir.dt.float16
OP = mybir.AluOpType
AF = mybir.ActivationFunctionType

NCORES = 8
P = 128
S = 65536            # box pairs per core
FTOT = S // P        # 512
NCHUNK = int(os.environ.get("KNCHUNK", "2"))
CLAMP = 4096.0
PI = float(np.pi)

_CACHE = {}
LAST_RESULTS = None

# flexible op -> engine (V=DVE, G=Pool); override: KASSIGN="nl8:G,dt:V"
ASSIGN = {"uv8m": "V", "nl8": "V", "hi8": "V", "dt": "V",
          "crossq": "V", "red": "V", "cp": "V", "dqh": "V"}
for _kv in os.environ.get("KASSIGN", "").split(","):
    if ":" in _kv:
        _k, _v = _kv.split(":")
        ASSIGN[_k.strip()] = _v.strip()


def _build_program(nchunk=NCHUNK):
    F = FTOT // nchunk
    RW = F * 5

    nc = bacc.Bacc("TRN2", target_bir_lowering=False, debug=False,
                   num_devices=NCORES)

    # register the pi/2 constant used as activation bias for cos-via-sin
    _ct = nc.alloc_sbuf_tensor("const-f32-halfpi", [128, 1], F32)
    nc.gpsimd.memset(_ct.ap(), PI / 2)
    nc.const_aps.aps[(F32, PI / 2)] = _ct.ap()
    nc.all_engine_barrier()

    b1 = nc.dram_tensor("b1", [S, 5], F32, kind="ExternalInput")
    b2 = nc.dram_tensor("b2", [S, 5], F32, kind="ExternalInput")
    iou = nc.dram_tensor("iou", [S], F32, kind="ExternalOutput")

    b1v = b1.ap().flatten().rearrange("(p q) -> p q", p=P)
    b2v = b2.ap().flatten().rearrange("(p q) -> p q", p=P)
    iouv = iou.ap().rearrange("(p q) -> p q", p=P)

    bufs = nchunk
    repeat = int(os.environ.get("KREPEAT", "1"))
    with TileContext(nc) as tc:
        with tc.tile_pool(name="rawp", bufs=bufs + 1) as rawp, \
             tc.tile_pool(name="pool", bufs=bufs) as pool:
            def emit_all():
                per_chunk = [_chunk_stages(nc, rawp, pool, b1v, b2v, iouv,
                                           c, F, RW)
                             for c in range(nchunk)]
                nst = len(per_chunk[0])
                offset = max(1, nst // nchunk)
                for i in range(nst + offset * (nchunk - 1)):
                    for ci in range(nchunk):
                        j = i - offset * ci
                        if 0 <= j < nst:
                            per_chunk[ci][j]()
            if repeat > 1:
                unroll = int(os.environ.get("KUNROLL", "16"))
                while repeat % unroll:
                    unroll -= 1
                with tc.For_i(0, repeat // unroll, 1, staggered_reset=True):
                    for _ in range(unroll):
                        emit_all()
            else:
                emit_all()
    nc.compile()
    return nc


def _chunk_stages(nc, rawp, pool, b1v, b2v, iouv, c, F, RW):
    """Build the chunk's instruction stream as a list of stage closures."""
    V, G, A = nc.vector, nc.gpsimd, nc.scalar
    sfx = f"_{c}"

    def t16(name, tag=None, w=1):
        return pool.tile([P, w * F], F16, name=name + sfx, tag=(tag or name))

    def t32(name, tag=None, w=1):
        return pool.tile([P, w * F], F32, name=name + sfx, tag=(tag or name))

    def rep(t, width):
        # [t | t] broadcast of a [P, width] AP, viewed [P, 2, width]
        return bass.AP(t.tensor, t.offset, [t.ap[0], [0, 2], [1, width]])

    def q(t):
        return t.rearrange("p (r f) -> p r f", r=2)

    class NS:
        pass

    n = NS()          # chunk-level tensors
    F2, F3, F4 = 2 * F, 3 * F, 4 * F

    class Ctx:
        pass

    def mk(pref, sgn1, sgn2, want_cross):
        o = Ctx()
        o.pref, o.sgn1, o.sgn2 = pref, sgn1, sgn2
        o.want_cross = want_cross
        return o

    # pass 1: box1 edges vs box2 (sr signs +,-); pass 2: signs -,+
    p1 = mk("p1", +1.0, -1.0, True)
    p2 = mk("p2", -1.0, +1.0, False)
    passes = [p1, p2]

    # ---------------- chunk-level stages -------------------------------
    def s_dma():
        n.raw1 = rawp.tile([P, RW], F32, name="raw1" + sfx, tag="raw1")
        n.raw2 = rawp.tile([P, RW], F32, name="raw2" + sfx, tag="raw2")
        nc.sync.dma_start(n.raw1[:], b1v[:, c * RW:(c + 1) * RW])
        nc.sync.dma_start(n.raw2[:], b2v[:, c * RW:(c + 1) * RW])
        n.x1, n.y1, n.w1, n.h1, n.a1 = (n.raw1[:, i:RW:5] for i in range(5))
        n.x2, n.y2, n.w2, n.h2, n.a2 = (n.raw2[:, i:RW:5] for i in range(5))
        p1.wa, p1.ha = n.w1, n.h1
        p2.wa, p2.ha = n.w2, n.h2

    def s_trig():
        n.da = t32("da")
        G.tensor_sub(n.da, n.a1, n.a2)
        n.sr = t32("sr")
        A.activation(n.sr, n.da, AF.Sin)
        n.ada = t32("ada")
        A.activation(n.ada, n.da, AF.Abs)
        n.cr = t32("cr")
        A.activation(n.cr, n.ada, AF.Sin, bias=PI / 2, scale=-1.0)  # cos(da)
        n.s2 = t16("s2")
        A.activation(n.s2, n.a2, AF.Sin)
        n.c2 = t16("c2")
        A.activation(n.c2, n.a2, AF.Sin, bias=PI / 2, scale=-1.0)   # cos(a2)
        n.s1 = t16("s1")
        A.activation(n.s1, n.a1, AF.Sin)
        n.c1 = t16("c1")
        A.activation(n.c1, n.a1, AF.Sin, bias=PI / 2, scale=-1.0)   # cos(a1)
        # pre-scaled trig (Pool has no scalar ops; fold signs/halves here)
        n.ncrh = t32("ncrh")
        A.activation(n.ncrh, n.cr, AF.Copy, scale=-0.5)
        n.psrh = t32("psrh")
        A.activation(n.psrh, n.sr, AF.Copy, scale=0.5)
        n.nsrh = t32("nsrh")
        A.activation(n.nsrh, n.sr, AF.Copy, scale=-0.5)

    def s_diff():
        n.dxp = t32("dxp")
        G.tensor_sub(n.dxp, n.x1, n.x2)
        n.dyp = t32("dyp")
        G.tensor_sub(n.dyp, n.y1, n.y2)
        n.dx16 = t16("dx16")
        V.tensor_copy(n.dx16, n.dxp)
        n.dy16 = t16("dy16")
        V.tensor_copy(n.dy16, n.dyp)
        # t = R2^T (c1-c2): box1 center in box2 frame (fp16, DVE 2x)
        n.e1 = t16("e1")
        V.tensor_mul(n.e1, n.dx16, n.c2)
        n.e2 = t16("e2")
        V.tensor_mul(n.e2, n.dy16, n.s2)
        n.txty = t16("txty", w=2)
        n.tx = n.txty[:, :F]
        V.tensor_add(n.tx, n.e1, n.e2)
        n.e3 = t16("e3", "e1")
        V.tensor_mul(n.e3, n.dy16, n.c2)
        n.e4 = t16("e4", "e2")
        V.tensor_mul(n.e4, n.dx16, n.s2)
        n.ty = n.txty[:, F:]
        V.tensor_sub(n.ty, n.e3, n.e4)

    def s_t2():
        # t2 = R1^T (c2-c1): t2x = -(dx*c1 + dy*s1); t2y = dx*s1 - dy*c1
        n.g1 = t16("g1", "e1")
        V.tensor_mul(n.g1, n.dx16, n.c1)
        n.g2 = t16("g2", "e2")
        V.tensor_mul(n.g2, n.dy16, n.s1)
        n.ng = t16("ng")                 # ng = t'x = dx*c1 + dy*s1
        V.tensor_add(n.ng, n.g1, n.g2)
        n.t2xy = t16("t2xy", w=2)
        n.t2x = n.t2xy[:, :F]
        V.tensor_scalar(out=n.t2x, in0=n.ng, scalar1=-1.0, scalar2=None,
                        op0=OP.mult)
        n.g3 = t16("g3", "e1")
        V.tensor_mul(n.g3, n.dx16, n.s1)
        n.g4 = t16("g4", "e2")
        V.tensor_mul(n.g4, n.dy16, n.c1)
        n.t2y = n.t2xy[:, F:]
        V.tensor_sub(n.t2y, n.g3, n.g4)  # t2y = -t'y
        n.t2yn = t16("t2yn")
        V.tensor_sub(n.t2yn, n.g4, n.g3)  # t'y

    def s_areas():
        n.area1 = t32("area1")
        G.tensor_mul(n.area1, n.w1, n.h1)
        n.area2 = t32("area2")
        G.tensor_mul(n.area2, n.w2, n.h2)
        n.ssum = t32("ssum")
        G.tensor_add(n.ssum, n.area1, n.area2)
        n.K1 = t16("K1")
        A.activation(n.K1, n.area1, AF.Copy, scale=0.5)
        n.ha2 = t16("ha2")
        A.activation(n.ha2, n.area2, AF.Copy, scale=0.5)
        # clip-box half extents, packed [0.5*w | 0.5*h] fp16
        n.wb2p = t16("wb2p", w=2)
        A.activation(n.wb2p[:, :F], n.w2, AF.Copy, scale=0.5)
        A.activation(n.wb2p[:, F:], n.h2, AF.Copy, scale=0.5)
        n.wb1p = t16("wb1p", w=2)
        A.activation(n.wb1p[:, :F], n.w1, AF.Copy, scale=0.5)
        A.activation(n.wb1p[:, F:], n.h1, AF.Copy, scale=0.5)
        # pass-1 cross linear terms: uv = [w1*t'y | -h1*t'x] fp16
        n.uvh = t16("uvh", w=2)
        V.tensor_mul(n.uvh[:, :F], n.t2yn, n.wb1p[:, :F])   # 0.5*w1*t'y
        V.tensor_mul(n.uvh[:, F:], n.t2x, n.wb1p[:, F:])    # -0.5*h1*t'x
        n.uv = t16("uv", w=2)
        V.tensor_scalar(out=n.uv, in0=n.uvh, scalar1=2.0, scalar2=None,
                        op0=OP.mult)
        p1.txty = n.txty
        p2.txty = n.t2xy
        p1.wbp = n.wb2p
        p2.wbp = n.wb1p

    # ---------------- per-pass stages ----------------------------------
    def stage_dqh(o):
        o.dQh = t32(o.pref + "dQh", o.pref + "dQh", w=4)
        s1t = n.psrh if o.sgn1 > 0 else n.nsrh
        s2t_ = n.psrh if o.sgn2 > 0 else n.nsrh
        ed = G if ASSIGN.get("dqh", "V") == "G" else V
        ed.tensor_mul(o.dQh[:, :F], n.ncrh, o.wa)
        ed.tensor_mul(o.dQh[:, F:F2], s1t, o.ha)
        ed.tensor_mul(o.dQh[:, F2:F3], s2t_, o.wa)
        ed.tensor_mul(o.dQh[:, F3:], n.ncrh, o.ha)

    def stage_recip(o):
        o.rQf = t32(o.pref + "rQf", o.pref + "rQf", w=4)
        V.reciprocal_approx_fast(out=o.rQf, in_=o.dQh)

    def stage_pq(o):
        nm = o.pref
        # PQuv = [P0 | nP1 | Q0 | nQ1]; P0 = -(d0+d1), Q0 = -(d2+d3)
        # Pool has no scalar/negate ops: compute the sums, negate on DVE.
        o.PQuv = t16(nm + "PQuv", nm + "tA", w=4)
        o.PQn2 = t16(nm + "PQn2", nm + "PQn2", w=2)
        ep = G if ASSIGN.get("pq", "V") == "G" else V
        ep.tensor_add(o.PQn2[:, :F], o.dQh[:, :F], o.dQh[:, F:F2])
        ep.tensor_sub(o.PQuv[:, F:F2], o.dQh[:, :F], o.dQh[:, F:F2])
        ep.tensor_add(o.PQn2[:, F:], o.dQh[:, F2:F3], o.dQh[:, F3:])
        ep.tensor_sub(o.PQuv[:, F3:], o.dQh[:, F2:F3], o.dQh[:, F3:])
        slot02 = bass.AP(o.PQuv.tensor, o.PQuv.offset,
                         [o.PQuv.ap[0], [F2, 2], [1, F]])
        V.tensor_scalar(out=slot02, in0=q(o.PQn2), scalar1=-1.0,
                        scalar2=None, op0=OP.mult)

    def stage_clamp(o):
        o.rq = t16(o.pref + "rq", o.pref + "tB", w=4)
        V.tensor_scalar(out=o.rq, in0=o.rQf, scalar1=CLAMP, scalar2=-CLAMP,
                        op0=OP.min, op1=OP.max)

    def stage_ptq(o):
        o.ptQ = t16(o.pref + "ptQ", o.pref + "tC", w=4)
        wbrep = bass.AP(o.wbp.tensor, o.wbp.offset,
                        [o.wbp.ap[0], [F, 2], [0, 2], [1, F]])
        V.tensor_tensor(o.ptQ.rearrange("p (j r f) -> p j r f", j=2, r=2),
                        wbrep,
                        o.rq.rearrange("p (j r f) -> p j r f", j=2, r=2),
                        OP.mult)

    def stage_wq(o):
        o.wQ = t16(o.pref + "wQ", o.pref + "tC", w=4)
        A.activation(o.wQ, o.ptQ, AF.Abs)

    def stage_uvq(o):
        nm = o.pref
        # UV8 = [PQuv + (tx,tx,ty,ty) | PQuv - (tx,tx,ty,ty)]
        # blocks (2F each): [u01 | v01 | -u23 | -v23]
        o.UV8 = t16(nm + "UV8", nm + "o8A", w=8)
        tt = o.txty
        ttrep = bass.AP(tt.tensor, tt.offset,
                        [tt.ap[0], [F, 2], [0, 2], [1, F]])
        qv = "p (j r f) -> p j r f"
        V.tensor_tensor(o.UV8[:, :F4].rearrange(qv, j=2, r=2),
                        o.PQuv.rearrange(qv, j=2, r=2), ttrep, OP.add)
        e = G if ASSIGN["uv8m"] == "G" else V
        e.tensor_tensor(o.UV8[:, F4:].rearrange(qv, j=2, r=2),
                        o.PQuv.rearrange(qv, j=2, r=2), ttrep, OP.subtract)

    def stage_mxy(o):
        nm = o.pref
        # m8 = UV8 * [ru | rv | ru | rv]
        o.m8 = t16(nm + "m8", nm + "o8B", w=8)
        V.tensor_tensor(q(o.m8), q(o.UV8), rep(o.rq, F4), OP.mult)

    def stage_nlhi(o):
        nm = o.pref
        # nl8 = m8 + [wQ | wQ]; hi8 = [wQ | wQ] - m8
        o.nl8 = t16(nm + "nl8", nm + "o8A", w=8)
        e1_ = G if ASSIGN["nl8"] == "G" else V
        e1_.tensor_tensor(q(o.nl8), q(o.m8), rep(o.wQ, F4), OP.add)
        o.hi8 = t16(nm + "hi8", nm + "o8B", w=8)
        e2_ = G if ASSIGN["hi8"] == "G" else V
        e2_.tensor_tensor(q(o.hi8), rep(o.wQ, F4), q(o.m8), OP.subtract)

    def stage_z(o):
        nm = o.pref
        # m1 = min(nl_u, nl_v); m2 = min(hi_u, hi_v)  (V: Pool min illegal)
        def blocks(t8, first):
            off = 0 if first else F2
            return bass.AP(t8.tensor, t8.offset + off,
                           [t8.ap[0], [F4, 2], [1, F2]])
        o.m1 = t16(nm + "m1", nm + "tD", w=4)
        V.tensor_tensor(o.m1.rearrange("p (r f) -> p r f", r=2),
                        blocks(o.nl8, True), blocks(o.nl8, False), OP.min)
        o.m2 = t16(nm + "m2", nm + "tE", w=4)
        V.tensor_tensor(o.m2.rearrange("p (r f) -> p r f", r=2),
                        blocks(o.hi8, True), blocks(o.hi8, False), OP.min)
        # z1 = min(m1, 0); z2 = min(m2, 2)   [t doubled]
        o.z1 = t16(nm + "z1", nm + "tA", w=4)
        V.tensor_scalar(out=o.z1, in0=o.m1, scalar1=0.0, scalar2=None,
                        op0=OP.min)
        o.z2 = t16(nm + "z2", nm + "tB", w=4)
        V.tensor_scalar(out=o.z2, in0=o.m2, scalar1=2.0, scalar2=None,
                        op0=OP.min)

    def stage_dt(o):
        o.dt = t16(o.pref + "dt", o.pref + "tC", w=4)
        e = G if ASSIGN["dt"] == "G" else V
        e.tensor_add(o.dt, o.z1, o.z2)

    def stage_tail(o):
        nm = o.pref
        ec = G if ASSIGN["crossq"] == "G" else V
        er = G if ASSIGN["red"] == "G" else V
        rdt = t16(nm + "rdt", nm + "tC", w=4)
        V.tensor_scalar(out=rdt, in0=o.dt, scalar1=0.0, scalar2=None,
                        op0=OP.max)
        if o.want_cross:
            crossQ = t16(nm + "crossQ", nm + "tA", w=4)
            ec.tensor_tensor(q(crossQ[:, :F2]), q(n.uv), rep(n.K1, F), OP.add)
            ec.tensor_tensor(q(crossQ[:, F2:]), rep(n.K1, F), q(n.uv),
                             OP.subtract)
            cc = t16(nm + "cc", nm + "tB", w=4)
            V.tensor_mul(cc, rdt, crossQ)
            s = t16(nm + "s", nm + "s", w=2)
            er.tensor_add(s, cc[:, :F2], cc[:, F2:])
            o.out = t16(nm + "accp", nm + "accp")
            er.tensor_add(o.out, s[:, :F], s[:, F:])
        else:
            s2t = t16(nm + "s2t", nm + "s2t", w=2)
            er.tensor_add(s2t, rdt[:, :F2], rdt[:, F2:])
            o.out = t16(nm + "sdt", nm + "sdt")
            er.tensor_add(o.out, s2t[:, :F], s2t[:, F:])

    def s_combine():
        accp1, sdt = p1.out, p2.out
        cp2 = t16("cp2")
        V.tensor_mul(cp2, sdt, n.ha2)
        acc = t16("acc")
        V.tensor_add(acc, accp1, cp2)
        inter = t32("inter")
        A.activation(inter, acc, AF.Abs, scale=0.25)
        union = t32("union", "ng")
        G.tensor_sub(union, n.ssum, inter)
        runion = t32("runion", "da")
        V.reciprocal_approx_fast(out=runion, in_=union)
        iouT = t32("iouT", "area1")
        G.tensor_mul(iouT, inter, runion)
        outq = os.environ.get("KOUT", "sp")
        if c == 0 or outq == "sp":
            nc.sync.dma_start(iouv[:, c * F:(c + 1) * F], iouT)
        else:
            G.dma_start(iouv[:, c * F:(c + 1) * F], iouT)

    def both(fn):
        def go():
            for o in passes:
                fn(o)
        return go

    return [s_dma, s_trig,
            both(stage_dqh), both(stage_recip),
            s_diff, both(stage_pq), both(stage_clamp),
            s_t2, s_areas,
            both(stage_ptq), both(stage_wq),
            both(stage_uvq), both(stage_mxy), both(stage_nlhi),
            both(stage_z), both(stage_dt), both(stage_tail),
            s_combine]


def _get_program():
    key = ("prog", NCHUNK, os.environ.get("KREPEAT", "1"),
           os.environ.get("KUNROLL", "16"), os.environ.get("KOUT", "sp"),
           os.environ.get("KASSIGN", ""))
    if key not in _CACHE:
        _CACHE[key] = _build_program(NCHUNK)
    return _CACHE[key]


def kernel(box1, box2, trace=False):
    global LAST_RESULTS
    b1 = np.ascontiguousarray(np.asarray(box1, dtype=np.float32))
    b2 = np.ascontiguousarray(np.asarray(box2, dtype=np.float32))
    B, N, C = b1.shape
    T = B * N
    assert T == NCORES * S and C == 5, (b1.shape,)
    b1f = b1.reshape(T, 5)
    b2f = b2.reshape(T, 5)

    in_maps = [
        {"b1": b1f[i * S:(i + 1) * S], "b2": b2f[i * S:(i + 1) * S]}
        for i in range(NCORES)
    ]
    nc = _get_program()
    res = run_bass_kernel_spmd(nc, in_maps, list(range(NCORES)), trace=trace)
    LAST_RESULTS = res
    out = np.concatenate([res.results[i]["iou"] for i in range(NCORES)])
    return out.reshape(B, N)
